# revision 34
# baseline (speedup 1.0000x reference)
"""BatchGAT (2-layer, 8-head GAT over 32 graphs of 512 nodes) on 8 TRN2 NeuronCores.

Data-parallel over the batch: each core processes 4 graphs. Per graph/layer the
masked attention matrix is built in transposed layout E^T[j, i] (j = neighbor on
partitions) and the aggregation runs TRANSPOSED on TensorE: lhsT = hp_aug
([128, 65] stationary, col 64 = const), rhs = E^T -> po[65, 1024] PSUM (two
heads = two banks), so the output lands FEATURE-major - exactly the lhsT layout
the next layer's projection needs (no transposes) - and the softmax denominator
falls out as PSUM row 64.

E-build exploits softmax row-scale invariance (divide row i by exp(0.2 s_i)):
E''*ed2 = max(q_i * ed_j, ed2_j) * adj with q = exp(0.8 s) broadcast rows and
ed = exp(d), ed2 = exp(0.2 d) per-partition scalars. Two engine paths, load
balanced:

 D-path (DVE): one dual-op tensor_scalar (mult-ptr, max-ptr) per head + one
   2-head-wide mask tensor_tensor (a few on GpSimd).

 A-path (ACT+PE): q*r - 1 + M via a K=1 outer-product matmul plus an
   identity-weight matmul of the additive mask adjM into PSUM; one ACT Relu
   evacuates it (relu(q r - 1 + M) = E'' - 1 on edges, 0 off edges); the
   missing "+1 * adj" rides the aggregation as a second matmul with rhs = adj
   and the ed2 factor via an lhsT fold (hps = hp_aug * ed2).

Normalization: den (PSUM row 64) is evacuated as part of a 65-row ACT copy
(free - ACT time is free-dim bound), gathered by small reshaping DMAs into a
[128, 32] tile so the (expensive, 8 cyc/elem) reciprocal runs wide, bounced via
DRAM and partition-broadcast into [128, 512] tiles for one tensor_tensor per
c-tile. The layer-1 head mean (/8) is folded into the aug column (8.0); the
head sum runs on TensorE with a [I64; I64] selector rhs, emitting node-major
output directly.
"""

import sys

if "/opt/trn_rl_repo" not in sys.path:
    sys.path.insert(0, "/opt/trn_rl_repo")

import numpy as np
import ml_dtypes

import concourse.bacc as bacc
import concourse.mybir as mybir
from concourse import tile
from concourse.bass_utils import run_bass_kernel_spmd
from concourse.alu_op_type import AluOpType

F32 = mybir.dt.float32
BF16 = mybir.dt.bfloat16
BF = ml_dtypes.bfloat16
AF = mybir.ActivationFunctionType

B, N, FIN, H, F = 32, 512, 64, 8, 64
NCORES = 8
G = B // NCORES          # graphs per core
NT = N // 128            # node tiles
C1 = H * F               # layer-1 input features (512)
W2 = 2 * N               # two heads side by side
BIG = 30000.0

_cached = {}


def _act_path(w, jt):
    """Which (wave, jtile) units build E on the ACT+PE path."""
    return ((w * NT + jt) * 5) % 16 < 5


def _mask_gps(w, jt):
    """Which D-path mask multiplies run on GpSimd instead of DVE."""
    return ((w * NT + jt) * 5) % 16 in (9, 12)


def _build():
    nc = bacc.Bacc("TRN2", target_bir_lowering=False, debug=False)

    xT = nc.dram_tensor("xT", [G, FIN, N], F32, kind="ExternalInput").ap()
    adjP = nc.dram_tensor("adjP", [G, N, N], BF16, kind="ExternalInput").ap()  # {0,1}
    adjM = nc.dram_tensor("adjM", [G, N, N], BF16, kind="ExternalInput").ap()  # {0,-BIG}
    ident = nc.dram_tensor("ident", [128, 128], BF16, kind="ExternalInput").ap()
    w0d = nc.dram_tensor("w0d", [FIN, F + 2 * H], F32, kind="ExternalInput").ap()
    w1d = nc.dram_tensor("w1d", [C1, F + 2 * H], BF16, kind="ExternalInput").ap()
    sel2 = nc.dram_tensor("sel2", [128, F], BF16, kind="ExternalInput").ap()
    out = nc.dram_tensor("out", [G, N, F], F32, kind="ExternalOutput").ap()

    with tile.TileContext(nc) as tc:
        _emit(nc, tc, xT, adjP, adjM, ident, w0d, w1d, sel2, out)
    nc.compile()
    return nc


def _emit(nc, tc, xT, adjP, adjM, ident, w0d, w1d, sel2, out):
    from contextlib import ExitStack

    ctx = ExitStack()
    with ctx:
        # weights: [W | W@a_dst | W@a_src] -> proj matmul yields [hp | d | s]
        wpool = ctx.enter_context(tc.tile_pool(name="weights", bufs=1))
        w0d_sb = wpool.tile([FIN, F + 2 * H], F32, tag="w0d")
        nc.sync.dma_start(w0d_sb[:], w0d[:])
        w1d_sb = wpool.tile([128, NT, F + 2 * H], BF16, tag="w1d")
        nc.sync.dma_start(w1d_sb[:], w1d.rearrange("(c p) f -> p c f", p=128))
        sel2_sb = wpool.tile([128, F], BF16, tag="sel2")
        nc.sync.dma_start(sel2_sb[:], sel2[:])
        id_sb = wpool.tile([128, 128], BF16, tag="ident")
        nc.sync.dma_start(id_sb[:], ident[:])
        neg1 = wpool.tile([128, 1], F32, tag="neg1")
        nc.vector.memset(neg1[:], -1.0)

        xt_pool = ctx.enter_context(tc.tile_pool(name="xt", bufs=3))
        adj_pool = ctx.enter_context(tc.tile_pool(name="adj", bufs=4 * NT))
        adjm_pool = ctx.enter_context(tc.tile_pool(name="adjm", bufs=4 * NT))
        hp_pool = ctx.enter_context(tc.tile_pool(name="hp", bufs=3 * NT))
        hps_pool = ctx.enter_context(tc.tile_pool(name="hps", bufs=12))
        dsc_pool = ctx.enter_context(tc.tile_pool(name="dscal", bufs=3 * NT))
        qrow_pool = ctx.enter_context(tc.tile_pool(name="qrow", bufs=2))
        qrf_pool = ctx.enter_context(tc.tile_pool(name="qrf", bufs=2))
        qbc_pool = ctx.enter_context(tc.tile_pool(name="qbc", bufs=5))
        u_pool = ctx.enter_context(tc.tile_pool(name="u", bufs=4))
        et_pool = ctx.enter_context(tc.tile_pool(name="et", bufs=8))
        zr_pool = ctx.enter_context(tc.tile_pool(name="zr", bufs=8))
        xe_pool = ctx.enter_context(tc.tile_pool(name="xe", bufs=2 * NT))
        scr_pool = ctx.enter_context(tc.tile_pool(name="scr", bufs=3))
        den_pool = ctx.enter_context(tc.tile_pool(name="den", bufs=4))
        rdbc_pool = ctx.enter_context(tc.tile_pool(name="rdbc", bufs=2 * NT))
        x1t_pool = ctx.enter_context(tc.tile_pool(name="x1t", bufs=4 * NT))
        x2t_pool = ctx.enter_context(tc.tile_pool(name="x2t", bufs=2 * NT))
        post_pool = ctx.enter_context(tc.tile_pool(name="post", bufs=2))
        out_pool = ctx.enter_context(tc.tile_pool(name="out", bufs=4))
        dbnc_pool = ctx.enter_context(tc.tile_pool(name="dbnc", bufs=3, space="DRAM"))

        ps_proj = ctx.enter_context(tc.tile_pool(name="ps_proj", bufs=1, space="PSUM"))
        ps_s = ctx.enter_context(tc.tile_pool(name="ps_s", bufs=1, space="PSUM"))
        ps_agg = ctx.enter_context(tc.tile_pool(name="ps_agg", bufs=2, space="PSUM"))
        ps_qr = ctx.enter_context(tc.tile_pool(name="ps_qr", bufs=2, space="PSUM"))

        graphs = {}

        def prologue(g, layer):
            st = {}
            if layer == 0:
                xt = xt_pool.tile([FIN, N], F32, tag="xt", name=f"xt_{g}")
                nc.gpsimd.dma_start(xt[:], xT[g])
                adj2, adjm_t = [], []
                for jt in range(NT):
                    a2 = adj_pool.tile([128, W2], BF16, tag="adj2",
                                       name=f"adj2_{g}_{jt}")
                    nc.gpsimd.dma_start(
                        a2[:, 0:N], adjP[g, jt * 128:(jt + 1) * 128, :])
                    nc.gpsimd.dma_start(
                        a2[:, N:W2], adjP[g, jt * 128:(jt + 1) * 128, :])
                    adj2.append(a2)
                    am = adjm_pool.tile([128, N], BF16, tag="adjm",
                                        name=f"adjm_{g}_{jt}")
                    nc.gpsimd.dma_start(
                        am[:], adjM[g, jt * 128:(jt + 1) * 128, :])
                    adjm_t.append(am)
                graphs[g] = dict(xt=xt, adj2=adj2, adjm=adjm_t)
            gs = graphs[g]
            xt = gs["xt"]
            x1t = gs.get("x1t")

            # ---- projections: [hp | d | s] per node tile ----
            hp_aug, ed_sc, ed2_sc = [], [], []
            for jt in range(NT):
                pp = ps_proj.tile([128, F + 2 * H], F32, tag="proj",
                                  name=f"pp_{g}_{layer}_{jt}")
                if layer == 0:
                    nc.tensor.matmul(
                        pp[:], xt[:, jt * 128:(jt + 1) * 128], w0d_sb[:],
                        start=True, stop=True)
                else:
                    for ct in range(NT):
                        nc.tensor.matmul(
                            pp[:], x1t[ct][:, jt * 128:(jt + 1) * 128],
                            w1d_sb[:, ct, :],
                            start=(ct == 0), stop=(ct == NT - 1))
                ha = hp_pool.tile([128, F + 1], BF16, tag="hp",
                                  name=f"ha_{g}_{layer}_{jt}")
                nc.scalar.copy(ha[:, 0:F], pp[:, 0:F])
                # aug column: 8.0 on layer 1 folds the head-mean into 1/den
                nc.gpsimd.memset(ha[:, F:F + 1], 1.0 if layer == 0 else 8.0)
                hp_aug.append(ha)
                ee = dsc_pool.tile([128, 2 * H], F32, tag="edsc",
                                   name=f"ee_{g}_{layer}_{jt}")
                nc.scalar.activation(ee[:, 0:H], pp[:, F:F + H], AF.Exp,
                                     scale=1.0)
                nc.scalar.activation(ee[:, H:2 * H], pp[:, F:F + H], AF.Exp,
                                     scale=0.2)
                ed_sc.append(ee[:, 0:H])
                ed2_sc.append(ee[:, H:2 * H])

            # s,d rows -> q = exp(0.8 s) bcast / r = exp(0.8 d) outer rows
            # (d rows at partitions 0..8, s rows at 32..40: engine partition
            # accesses must start 32-aligned)
            psd = ps_s.tile([32 + H, N], F32, tag="s", name=f"psd_{g}_{layer}")
            if layer == 0:
                nc.tensor.matmul(psd[0:H, :], w0d_sb[:, F:F + H], xt[:],
                                 start=True, stop=True)
                nc.tensor.matmul(psd[32:32 + H, :],
                                 w0d_sb[:, F + H:F + 2 * H], xt[:],
                                 start=True, stop=True)
            else:
                for ct in range(NT):
                    nc.tensor.matmul(
                        psd[0:H, :], w1d_sb[:, ct, F:F + H], x1t[ct][:],
                        start=(ct == 0), stop=(ct == NT - 1))
                for ct in range(NT):
                    nc.tensor.matmul(
                        psd[32:32 + H, :], w1d_sb[:, ct, F + H:F + 2 * H],
                        x1t[ct][:], start=(ct == 0), stop=(ct == NT - 1))
            qd = qrow_pool.tile([32 + H, N], BF16, tag="qd",
                                name=f"qd_{g}_{layer}")
            nc.scalar.activation(qd[0:H, :], psd[32:32 + H, :], AF.Exp,
                                 scale=0.8)
            nc.scalar.activation(qd[32:32 + H, :], psd[0:H, :], AF.Exp,
                                 scale=0.8)
            qdram = dbnc_pool.tile([2 * H, N], BF16, tag="qdram",
                                   name=f"qdram_{g}_{layer}")
            nc.sync.dma_start(qdram[0:H, :], qd[0:H, :])
            nc.sync.dma_start(qdram[H:2 * H, :], qd[32:32 + H, :])
            # flat single-partition [q-flat | r-flat] for base-0 K=1 outer
            # products on the A-path
            qrf = qrf_pool.tile([1, 2 * H * N], BF16, tag="qrf",
                                name=f"qrf_{g}_{layer}")
            nc.sync.dma_start(qrf[:],
                              qdram.rearrange("h f -> () (h f)"))
            q_bc = []
            for w in range(NT):
                if all(_act_path(w, jt) for jt in range(NT)):
                    q_bc.append(None)
                    continue
                qb = qbc_pool.tile([128, W2], BF16, tag="qbc",
                                   name=f"qbc_{g}_{layer}_{w}")
                nc.gpsimd.dma_start(
                    qb[:],
                    qdram[2 * w:2 * w + 2, :]
                    .rearrange("a f -> () (a f)").partition_broadcast(128))
                q_bc.append(qb)

            st.update(hp_aug=hp_aug, ed_sc=ed_sc, ed2_sc=ed2_sc, q_bc=q_bc,
                      qrf=qrf)
            return st

        def unit_ctx(g, layer, st):
            return dict(
                st=st, xe=[None] * NT, rdbc=[None] * NT, x2t=[None] * NT,
                hps={},
                den128=den_pool.tile([128, 32], BF16, tag="den128",
                                     name=f"den128_{g}_{layer}"),
                rd128=den_pool.tile([128, 32], BF16, tag="rd128",
                                    name=f"rd_{g}_{layer}"),
                dnd=dbnc_pool.tile([H, N], BF16, tag="dnd",
                                   name=f"dnd_{g}_{layer}"),
                rddram=dbnc_pool.tile([H, N], BF16, tag="rddram",
                                      name=f"rddram_{g}_{layer}"),
            )

        def wave_build(g, layer, w, cx):
            gs = graphs[g]
            adj2, adjm_t = gs["adj2"], gs["adjm"]
            st = cx["st"]
            hp_aug, ed_sc, ed2_sc = st["hp_aug"], st["ed_sc"], st["ed2_sc"]
            q_bc, qrf = st["q_bc"], st["qrf"]

            def hps_for(h, jt):
                if (h, jt) not in cx["hps"]:
                    hs = hps_pool.tile([128, F + 1], BF16, tag="hps",
                                       name=f"hps_{g}_{layer}_{h}_{jt}")
                    nc.vector.tensor_scalar(
                        hs[:], hp_aug[jt][:], ed2_sc[jt][:, h:h + 1], None,
                        AluOpType.mult)
                    cx["hps"][(h, jt)] = hs
                return cx["hps"][(h, jt)]

            h0, h1 = 2 * w, 2 * w + 1
            mm = {h0: [], h1: []}
            for jt in range(NT):
                if _act_path(w, jt):
                    for hh in (h0, h1):
                        pq = ps_qr.tile([128, N], F32, tag="qr",
                                        name=f"pq_{g}_{layer}_{hh}_{jt}")
                        # q*r via K=1 outer product, + additive mask
                        nc.tensor.matmul(
                            pq[:],
                            qrf[0:1, (H + hh) * N + jt * 128:
                                (H + hh) * N + (jt + 1) * 128],
                            qrf[0:1, hh * N:(hh + 1) * N],
                            start=True, stop=False)
                        nc.tensor.matmul(
                            pq[:], id_sb[:], adjm_t[jt][:],
                            start=False, stop=True)
                        z = zr_pool.tile([128, N], BF16, tag="zr",
                                         name=f"zr_{g}_{layer}_{hh}_{jt}")
                        nc.scalar.activation(z[:], pq[:], AF.Relu,
                                             bias=neg1[:])
                        hs = hps_for(hh, jt)
                        mm[hh].append((z[:], hs[:]))
                        mm[hh].append((adj2[jt][:, 0:N], hs[:]))
                else:
                    u2 = u_pool.tile([128, W2], BF16, tag="u",
                                     name=f"u_{g}_{layer}_{w}_{jt}")
                    # E''*ed2 = max(q_i*ed_j, ed2_j): one dual-op TS/head
                    nc.vector.tensor_scalar(
                        u2[:, 0:N], q_bc[w][:, 0:N],
                        ed_sc[jt][:, h0:h0 + 1], ed2_sc[jt][:, h0:h0 + 1],
                        AluOpType.mult, AluOpType.max)
                    nc.vector.tensor_scalar(
                        u2[:, N:W2], q_bc[w][:, N:W2],
                        ed_sc[jt][:, h1:h1 + 1], ed2_sc[jt][:, h1:h1 + 1],
                        AluOpType.mult, AluOpType.max)
                    et = et_pool.tile([128, W2], BF16, tag="et",
                                      name=f"et_{g}_{layer}_{w}_{jt}")
                    eng = nc.gpsimd if _mask_gps(w, jt) else nc.vector
                    eng.tensor_tensor(et[:], u2[:], adj2[jt][:],
                                      AluOpType.mult)
                    mm[h0].append((et[:, 0:N], hp_aug[jt][:]))
                    mm[h1].append((et[:, N:W2], hp_aug[jt][:]))

            cx.setdefault("mm", {})[w] = mm

        def wave_agg(g, layer, w, cx):
            h0, h1 = 2 * w, 2 * w + 1
            mm = cx["mm"].pop(w)
            po = ps_agg.tile([F + 1, W2], F32, tag="agg",
                             name=f"po_{g}_{layer}_{w}")
            for k, hh in enumerate((h0, h1)):
                sl = slice(k * N, (k + 1) * N)
                for i, (rhs, lhsT) in enumerate(mm[hh]):
                    nc.tensor.matmul(po[:, sl], lhsT, rhs,
                                     start=(i == 0),
                                     stop=(i == len(mm[hh]) - 1))

            # evac: 65 rows (den rides along as row 64, free on ACT)
            dnd = cx["dnd"]
            xew = xe_pool.tile([128, N], BF16, tag="xe",
                               name=f"xe_{g}_{layer}_{w}")
            nc.scalar.copy(xew[0:F + 1, :], po[0:F + 1, 0:N])
            nc.sync.dma_start(dnd[h0:h0 + 1, :], xew[F:F + 1, :])
            scr = scr_pool.tile([F + 1, N], BF16, tag="scr",
                                name=f"scr_{g}_{layer}_{w}")
            nc.scalar.copy(scr[:], po[0:F + 1, N:W2])
            nc.sync.dma_start(dnd[h1:h1 + 1, :], scr[F:F + 1, :])
            nc.sync.dma_start(xew[F:128, :], scr[0:F, :])
            nc.sync.dma_start(
                cx["den128"][32 * w:32 * w + 32, :],
                dnd[h0:h0 + 2, :].rearrange("h (p a) -> (h p) a", a=32))
            cx["xe"][w] = xew

        def wave_mid(g, layer, w, cx):
            # reciprocal + DRAM bounce + broadcast (issued 2 waves after
            # build so the den DMA chain has landed; consumed 2 waves later)
            rd128, den128 = cx["rd128"], cx["den128"]
            sl = slice(32 * w, 32 * w + 32)
            with nc.allow_low_precision(reason="1/den in bf16 is ample"):
                nc.vector.reciprocal(rd128[sl, :], den128[sl, :])
            nc.sync.dma_start(
                cx["rddram"][2 * w:2 * w + 2, :]
                .rearrange("h (p a) -> (h p) a", a=32), rd128[sl, :])
            rb = rdbc_pool.tile([128, N], BF16, tag="rdbc",
                                name=f"rdbc_{g}_{layer}_{w}")
            nc.sync.dma_start(
                rb[0:F, :],
                cx["rddram"][2 * w:2 * w + 1, :].partition_broadcast(F))
            nc.sync.dma_start(
                rb[F:128, :],
                cx["rddram"][2 * w + 1:2 * w + 2, :].partition_broadcast(F))
            cx["rdbc"][w] = rb

        def wave_post(g, layer, w, cx):
            xew, rb = cx["xe"][w], cx["rdbc"][w]
            if layer == 0:
                if "x1t" not in graphs[g]:
                    graphs[g]["x1t"] = [None] * NT
                x1 = x1t_pool.tile([128, N], BF16, tag="x1t",
                                   name=f"x1t_{g}_{w}")
                graphs[g]["x1t"][w] = x1
                xn = post_pool.tile([128, N], BF16, tag="xn",
                                    name=f"xn_{g}_{w}")
                nc.vector.tensor_tensor(xn[:], xew[:], rb[:],
                                        AluOpType.mult)
                te = post_pool.tile([128, N], BF16, tag="te",
                                    name=f"te_{g}_{w}")
                nc.scalar.activation(te[:], xn[:], AF.Exp)
                rl = post_pool.tile([128, N], BF16, tag="rl",
                                    name=f"rl_{g}_{w}")
                nc.scalar.activation(rl[:], xn[:], AF.Relu)
                # elu(x) = min(relu(x), exp(x) - 1)
                nc.vector.scalar_tensor_tensor(
                    x1[:], te[:], -1.0, rl[:],
                    AluOpType.add, AluOpType.min)
            else:
                xn = x2t_pool.tile([128, N], BF16, tag="x2t",
                                   name=f"x2t_{g}_{w}")
                nc.vector.tensor_tensor(xn[:], xew[:], rb[:],
                                        AluOpType.mult)
                cx["x2t"][w] = xn

        def unit_final(g, layer, cx):
            if layer == 0:
                return
            x2t = cx["x2t"]
            oo = out_pool.tile([128, NT * F], F32, tag="oo",
                               name=f"oo_{g}")
            for it in range(NT):
                po2 = ps_qr.tile([128, F], F32, tag="qr",
                                 name=f"po2_{g}_{it}")
                for ct in range(NT):
                    nc.tensor.matmul(
                        po2[:], x2t[ct][:, it * 128:(it + 1) * 128],
                        sel2_sb[:], start=(ct == 0), stop=(ct == NT - 1))
                nc.scalar.copy(oo[:, it * F:(it + 1) * F], po2[:])
            nc.sync.dma_start(
                out[g].rearrange("(it p) f -> p it f", p=128),
                oo[:].rearrange("p (it f) -> p it f", it=NT))

        # wave-granular software pipelining:
        #   build(j) | agg(j-1) | mid(j-2) | post(j-4)
        # The one-wave E-build -> aggregation stagger lets agg matmuls find
        # their rhs tiles resident (back-to-back PE streaming instead of
        # per-matmul drain stalls); mid/post deferral keeps the
        # den/reciprocal DMA chains from parking an in-order engine on a
        # semaphore wait. Unit k+1's prologue lands inside unit k.
        U = [(0, 0), (1, 0), (2, 0), (0, 1), (3, 0), (1, 1), (2, 1), (3, 1)]
        from collections import deque
        aggq, midq, postq = deque(), deque(), deque()
        pending = {U[0]: prologue(*U[0])}
        for i, u in enumerate(U):
            g, layer = u
            cx = unit_ctx(g, layer, pending.pop(u))
            for w in range(NT):
                wave_build(g, layer, w, cx)
                aggq.append((g, layer, w, cx))
                if len(aggq) > 1:
                    jb = aggq.popleft()
                    wave_agg(*jb)
                    midq.append(jb)
                if len(midq) > 1:
                    jb = midq.popleft()
                    wave_mid(*jb)
                    postq.append(jb)
                if len(postq) > 1:
                    jb = postq.popleft()
                    wave_post(*jb)
                    if jb[2] == NT - 1:
                        unit_final(jb[0], jb[1], jb[3])
                if w == 0 and i + 1 < len(U):
                    pending[U[i + 1]] = prologue(*U[i + 1])
        while aggq:
            jb = aggq.popleft()
            wave_agg(*jb)
            midq.append(jb)
        while midq:
            jb = midq.popleft()
            wave_mid(*jb)
            postq.append(jb)
        while postq:
            jb = postq.popleft()
            wave_post(*jb)
            if jb[2] == NT - 1:
                unit_final(jb[0], jb[1], jb[3])


def _get_nc():
    if "nc" not in _cached:
        _cached["nc"] = _build()
    return _cached["nc"]


def _prep_inputs(x, adj, W0, a_src0, a_dst0, W1, a_src1, a_dst1):
    x = np.asarray(x, np.float32)
    adj = np.array(adj, np.float32, copy=True)
    idx = np.arange(N)
    adj[:, idx, idx] = 1.0  # self loops (reference mutates adj the same way)
    xT = np.ascontiguousarray(x.transpose(0, 2, 1))          # [B, 64, 512]
    adjPf = np.where(adj > 0, np.float32(1), np.float32(0)).astype(BF)
    adjMf = np.where(adj > 0, np.float32(0), np.float32(-BIG)).astype(BF)
    identf = np.eye(128, dtype=np.float32).astype(BF)
    W0 = np.asarray(W0, np.float32)
    W1 = np.asarray(W1, np.float32)
    # column layout: [W | W@a_dst | W@a_src] -> pp = [hp | d | s]
    w0d = np.concatenate(
        [W0, W0 @ np.asarray(a_dst0, np.float32),
         W0 @ np.asarray(a_src0, np.float32)], axis=1)
    w1d = np.concatenate(
        [W1, W1 @ np.asarray(a_dst1, np.float32),
         W1 @ np.asarray(a_src1, np.float32)], axis=1).astype(BF)
    sel2f = np.tile(np.eye(F, dtype=np.float32), (2, 1)).astype(BF)
    in_maps = []
    for c in range(NCORES):
        sl = slice(c * G, (c + 1) * G)
        in_maps.append(dict(
            xT=np.ascontiguousarray(xT[sl]),
            adjP=np.ascontiguousarray(adjPf[sl]),
            adjM=np.ascontiguousarray(adjMf[sl]),
            ident=identf, w0d=w0d, w1d=w1d, sel2=sel2f,
        ))
    return in_maps


def run(inputs, **kw):
    """Build+run; returns (output [B,N,F] float32, BassKernelResults)."""
    nc = _get_nc()
    in_maps = _prep_inputs(
        inputs["x"], inputs["adj"], inputs["W0"], inputs["a_src0"],
        inputs["a_dst0"], inputs["W1"], inputs["a_src1"], inputs["a_dst1"])
    res = run_bass_kernel_spmd(nc, in_maps, list(range(NCORES)), **kw)
    outs = [res.results[c]["out"].reshape(G, N, F) for c in range(NCORES)]
    return np.concatenate(outs, axis=0).astype(np.float32), res


def kernel(**inputs):
    out, _ = run(inputs)
    return out


# revision 36
# speedup vs baseline: 1.1749x; 1.1749x over previous
"""BatchGAT (2-layer, 8-head GAT over 32 graphs of 512 nodes) on 8 TRN2 NeuronCores.

Data-parallel over the batch: each core processes 4 graphs. Per graph/layer the
masked attention matrix is built in transposed layout E^T[j, i] (j = neighbor on
partitions) and the aggregation runs TRANSPOSED on TensorE: lhsT = hp_aug
([128, 65] stationary, col 64 = const), rhs = E^T -> po[65, 1024] PSUM (two
heads = two banks), so the output lands FEATURE-major - exactly the lhsT layout
the next layer's projection needs (no transposes) - and the softmax denominator
falls out as PSUM row 64.

E-build exploits softmax row-scale invariance (divide row i by exp(0.2 s_i)):
E''*ed2 = max(q_i * ed_j, ed2_j) * adj with q = exp(0.8 s) broadcast rows and
ed = exp(d), ed2 = exp(0.2 d) per-partition scalars. Two engine paths, load
balanced:

 D-path (DVE): one dual-op tensor_scalar (mult-ptr, max-ptr) per head + one
   2-head-wide mask tensor_tensor (a few on GpSimd).

 A-path (ACT+PE): q*r - 1 + M via a K=1 outer-product matmul plus an
   identity-weight matmul of the additive mask adjM into PSUM; one ACT Relu
   evacuates it (relu(q r - 1 + M) = E'' - 1 on edges, 0 off edges); the
   missing "+1 * adj" rides the aggregation as a second matmul with rhs = adj
   and the ed2 factor via an lhsT fold (hps = hp_aug * ed2).

Normalization: den (PSUM row 64) is evacuated as part of a 65-row ACT copy
(free - ACT time is free-dim bound), gathered by small reshaping DMAs into a
[128, 32] tile so the (expensive, 8 cyc/elem) reciprocal runs wide, bounced via
DRAM and partition-broadcast into [128, 512] tiles for one tensor_tensor per
c-tile. The layer-1 head mean (/8) is folded into the aug column (8.0); the
head sum runs on TensorE with a [I64; I64] selector rhs, emitting node-major
output directly.
"""

import sys

if "/opt/trn_rl_repo" not in sys.path:
    sys.path.insert(0, "/opt/trn_rl_repo")

import numpy as np
import ml_dtypes

import concourse.bacc as bacc
import concourse.mybir as mybir
from concourse import tile
from concourse.bass_utils import run_bass_kernel_spmd
from concourse.alu_op_type import AluOpType

F32 = mybir.dt.float32
BF16 = mybir.dt.bfloat16
BF = ml_dtypes.bfloat16
AF = mybir.ActivationFunctionType

B, N, FIN, H, F = 32, 512, 64, 8, 64
NCORES = 8
G = B // NCORES          # graphs per core
NT = N // 128            # node tiles
C1 = H * F               # layer-1 input features (512)
W2 = 2 * N               # two heads side by side
BIG = 30000.0

_cached = {}


def _act_path(w, jt):
    """Which (wave, jtile) units build E on the ACT+PE path."""
    return ((w * NT + jt) * 5) % 16 < 5


def _mask_gps(w, jt):
    """Which D-path mask multiplies run on GpSimd instead of DVE."""
    return ((w * NT + jt) * 5) % 16 in (9, 12)


def _build():
    nc = bacc.Bacc("TRN2", target_bir_lowering=False, debug=False)

    xT = nc.dram_tensor("xT", [G, FIN, N], F32, kind="ExternalInput").ap()
    adjP = nc.dram_tensor("adjP", [G, N, N], BF16, kind="ExternalInput").ap()  # {0,1}
    adjM = nc.dram_tensor("adjM", [G, N, N], BF16, kind="ExternalInput").ap()  # {0,-BIG}
    ident = nc.dram_tensor("ident", [128, 128], BF16, kind="ExternalInput").ap()
    w0d = nc.dram_tensor("w0d", [FIN, F + 2 * H], F32, kind="ExternalInput").ap()
    w1d = nc.dram_tensor("w1d", [C1, F + 2 * H], BF16, kind="ExternalInput").ap()
    sel2 = nc.dram_tensor("sel2", [128, F], BF16, kind="ExternalInput").ap()
    out = nc.dram_tensor("out", [G, N, F], F32, kind="ExternalOutput").ap()

    with tile.TileContext(nc) as tc:
        _emit(nc, tc, xT, adjP, adjM, ident, w0d, w1d, sel2, out)
    nc.compile()
    return nc


def _emit(nc, tc, xT, adjP, adjM, ident, w0d, w1d, sel2, out):
    from contextlib import ExitStack

    ctx = ExitStack()
    with ctx:
        # weights: [W | W@a_dst | W@a_src] -> proj matmul yields [hp | d | s]
        wpool = ctx.enter_context(tc.tile_pool(name="weights", bufs=1))
        w0d_sb = wpool.tile([FIN, F + 2 * H], F32, tag="w0d")
        nc.sync.dma_start(w0d_sb[:], w0d[:])
        w1d_sb = wpool.tile([128, NT, F + 2 * H], BF16, tag="w1d")
        nc.sync.dma_start(w1d_sb[:], w1d.rearrange("(c p) f -> p c f", p=128))
        sel2_sb = wpool.tile([128, F], BF16, tag="sel2")
        nc.sync.dma_start(sel2_sb[:], sel2[:])
        id_sb = wpool.tile([128, 128], BF16, tag="ident")
        nc.sync.dma_start(id_sb[:], ident[:])
        neg1 = wpool.tile([128, 1], F32, tag="neg1")
        nc.vector.memset(neg1[:], -1.0)

        xt_pool = ctx.enter_context(tc.tile_pool(name="xt", bufs=3))
        adj_pool = ctx.enter_context(tc.tile_pool(name="adj", bufs=4 * NT))
        adjm_pool = ctx.enter_context(tc.tile_pool(name="adjm", bufs=4 * NT))
        hp_pool = ctx.enter_context(tc.tile_pool(name="hp", bufs=3 * NT))
        hps_pool = ctx.enter_context(tc.tile_pool(name="hps", bufs=2 * NT))
        dsc_pool = ctx.enter_context(tc.tile_pool(name="dscal", bufs=3 * NT))
        qrow_pool = ctx.enter_context(tc.tile_pool(name="qrow", bufs=2))
        qrf_pool = ctx.enter_context(tc.tile_pool(name="qrf", bufs=2))
        qbc_pool = ctx.enter_context(tc.tile_pool(name="qbc", bufs=6))
        u_pool = ctx.enter_context(tc.tile_pool(name="u", bufs=5))
        et_pool = ctx.enter_context(tc.tile_pool(name="et", bufs=6))
        zr_pool = ctx.enter_context(tc.tile_pool(name="zr", bufs=6))
        xe_pool = ctx.enter_context(tc.tile_pool(name="xe", bufs=2 * NT))
        scr_pool = ctx.enter_context(tc.tile_pool(name="scr", bufs=4))
        den_pool = ctx.enter_context(tc.tile_pool(name="den", bufs=4))
        rdbc_pool = ctx.enter_context(tc.tile_pool(name="rdbc", bufs=2 * NT))
        x1t_pool = ctx.enter_context(tc.tile_pool(name="x1t", bufs=4 * NT))
        x2t_pool = ctx.enter_context(tc.tile_pool(name="x2t", bufs=2 * NT))
        post_pool = ctx.enter_context(tc.tile_pool(name="post", bufs=3))
        out_pool = ctx.enter_context(tc.tile_pool(name="out", bufs=4))
        dbnc_pool = ctx.enter_context(tc.tile_pool(name="dbnc", bufs=3, space="DRAM"))

        ps_proj = ctx.enter_context(tc.tile_pool(name="ps_proj", bufs=1, space="PSUM"))
        ps_s = ctx.enter_context(tc.tile_pool(name="ps_s", bufs=1, space="PSUM"))
        ps_agg = ctx.enter_context(tc.tile_pool(name="ps_agg", bufs=2, space="PSUM"))
        ps_qr = ctx.enter_context(tc.tile_pool(name="ps_qr", bufs=2, space="PSUM"))

        graphs = {}

        def prologue(g, layer):
            st = {}
            if layer == 0:
                xt = xt_pool.tile([FIN, N], F32, tag="xt", name=f"xt_{g}")
                nc.gpsimd.dma_start(xt[:], xT[g])
                adj2, adjm_t = [], []
                for jt in range(NT):
                    a2 = adj_pool.tile([128, W2], BF16, tag="adj2",
                                       name=f"adj2_{g}_{jt}")
                    nc.gpsimd.dma_start(
                        a2[:, 0:N], adjP[g, jt * 128:(jt + 1) * 128, :])
                    nc.gpsimd.dma_start(
                        a2[:, N:W2], adjP[g, jt * 128:(jt + 1) * 128, :])
                    adj2.append(a2)
                    am = adjm_pool.tile([128, N], BF16, tag="adjm",
                                        name=f"adjm_{g}_{jt}")
                    nc.gpsimd.dma_start(
                        am[:], adjM[g, jt * 128:(jt + 1) * 128, :])
                    adjm_t.append(am)
                graphs[g] = dict(xt=xt, adj2=adj2, adjm=adjm_t)
            gs = graphs[g]
            xt = gs["xt"]
            x1t = gs.get("x1t")

            # ---- projections: [hp | d | s] per node tile ----
            hp_aug, ed_sc, ed2_sc = [], [], []
            for jt in range(NT):
                pp = ps_proj.tile([128, F + 2 * H], F32, tag="proj",
                                  name=f"pp_{g}_{layer}_{jt}")
                if layer == 0:
                    nc.tensor.matmul(
                        pp[:], xt[:, jt * 128:(jt + 1) * 128], w0d_sb[:],
                        start=True, stop=True)
                else:
                    for ct in range(NT):
                        nc.tensor.matmul(
                            pp[:], x1t[ct][:, jt * 128:(jt + 1) * 128],
                            w1d_sb[:, ct, :],
                            start=(ct == 0), stop=(ct == NT - 1))
                ha = hp_pool.tile([128, F + 1], BF16, tag="hp",
                                  name=f"ha_{g}_{layer}_{jt}")
                nc.scalar.copy(ha[:, 0:F], pp[:, 0:F])
                # aug column: 8.0 on layer 1 folds the head-mean into 1/den
                nc.gpsimd.memset(ha[:, F:F + 1], 1.0 if layer == 0 else 8.0)
                hp_aug.append(ha)
                ee = dsc_pool.tile([128, 2 * H], F32, tag="edsc",
                                   name=f"ee_{g}_{layer}_{jt}")
                nc.scalar.activation(ee[:, 0:H], pp[:, F:F + H], AF.Exp,
                                     scale=1.0)
                nc.scalar.activation(ee[:, H:2 * H], pp[:, F:F + H], AF.Exp,
                                     scale=0.2)
                ed_sc.append(ee[:, 0:H])
                ed2_sc.append(ee[:, H:2 * H])

            # s,d rows -> q = exp(0.8 s) bcast / r = exp(0.8 d) outer rows
            # (d rows at partitions 0..8, s rows at 32..40: engine partition
            # accesses must start 32-aligned)
            psd = ps_s.tile([32 + H, N], F32, tag="s", name=f"psd_{g}_{layer}")
            if layer == 0:
                nc.tensor.matmul(psd[0:H, :], w0d_sb[:, F:F + H], xt[:],
                                 start=True, stop=True)
                nc.tensor.matmul(psd[32:32 + H, :],
                                 w0d_sb[:, F + H:F + 2 * H], xt[:],
                                 start=True, stop=True)
            else:
                for ct in range(NT):
                    nc.tensor.matmul(
                        psd[0:H, :], w1d_sb[:, ct, F:F + H], x1t[ct][:],
                        start=(ct == 0), stop=(ct == NT - 1))
                for ct in range(NT):
                    nc.tensor.matmul(
                        psd[32:32 + H, :], w1d_sb[:, ct, F + H:F + 2 * H],
                        x1t[ct][:], start=(ct == 0), stop=(ct == NT - 1))
            qd = qrow_pool.tile([32 + H, N], BF16, tag="qd",
                                name=f"qd_{g}_{layer}")
            nc.scalar.activation(qd[0:H, :], psd[32:32 + H, :], AF.Exp,
                                 scale=0.8)
            nc.scalar.activation(qd[32:32 + H, :], psd[0:H, :], AF.Exp,
                                 scale=0.8)
            qdram = dbnc_pool.tile([2 * H, N], BF16, tag="qdram",
                                   name=f"qdram_{g}_{layer}")
            nc.sync.dma_start(qdram[0:H, :], qd[0:H, :])
            nc.sync.dma_start(qdram[H:2 * H, :], qd[32:32 + H, :])
            # flat single-partition [q-flat | r-flat] for base-0 K=1 outer
            # products on the A-path
            qrf = qrf_pool.tile([1, 2 * H * N], BF16, tag="qrf",
                                name=f"qrf_{g}_{layer}")
            nc.sync.dma_start(qrf[:],
                              qdram.rearrange("h f -> () (h f)"))
            q_bc = []
            for w in range(NT):
                if all(_act_path(w, jt) for jt in range(NT)):
                    q_bc.append(None)
                    continue
                qb = qbc_pool.tile([128, W2], BF16, tag="qbc",
                                   name=f"qbc_{g}_{layer}_{w}")
                nc.gpsimd.dma_start(
                    qb[:],
                    qdram[2 * w:2 * w + 2, :]
                    .rearrange("a f -> () (a f)").partition_broadcast(128))
                q_bc.append(qb)

            st.update(hp_aug=hp_aug, ed_sc=ed_sc, ed2_sc=ed2_sc, q_bc=q_bc,
                      qrf=qrf)
            return st

        def main(g, layer, st):
            gs = graphs[g]
            adj2, adjm_t = gs["adj2"], gs["adjm"]
            hp_aug, ed_sc, ed2_sc = st["hp_aug"], st["ed_sc"], st["ed2_sc"]
            q_bc, qrf = st["q_bc"], st["qrf"]

            den128 = den_pool.tile([128, 32], BF16, tag="den128",
                                   name=f"den128_{g}_{layer}")
            dnd = dbnc_pool.tile([H, N], BF16, tag="dnd",
                                 name=f"dnd_{g}_{layer}")
            xe = []
            hps_cache = {}

            def hps_for(h, jt):
                if (h, jt) not in hps_cache:
                    hs = hps_pool.tile([128, F + 1], BF16, tag="hps",
                                       name=f"hps_{g}_{layer}_{h}_{jt}")
                    nc.vector.tensor_scalar(
                        hs[:], hp_aug[jt][:], ed2_sc[jt][:, h:h + 1], None,
                        AluOpType.mult)
                    hps_cache[(h, jt)] = hs
                return hps_cache[(h, jt)]

            # ---- per wave (2 heads): E build + aggregation + evac ----
            for w in range(NT):
                h0, h1 = 2 * w, 2 * w + 1
                po = ps_agg.tile([F + 1, W2], F32, tag="agg",
                                 name=f"po_{g}_{layer}_{w}")
                # (rhs, lhsT, half) matmul operand lists per head
                mm = {h0: [], h1: []}
                for jt in range(NT):
                    if _act_path(w, jt):
                        for hh in (h0, h1):
                            pq = ps_qr.tile([128, N], F32, tag="qr",
                                            name=f"pq_{g}_{layer}_{hh}_{jt}")
                            # q*r via K=1 outer product, + additive mask
                            nc.tensor.matmul(
                                pq[:],
                                qrf[0:1, (H + hh) * N + jt * 128:
                                    (H + hh) * N + (jt + 1) * 128],
                                qrf[0:1, hh * N:(hh + 1) * N],
                                start=True, stop=False)
                            nc.tensor.matmul(
                                pq[:], id_sb[:], adjm_t[jt][:],
                                start=False, stop=True)
                            z = zr_pool.tile([128, N], BF16, tag="zr",
                                             name=f"zr_{g}_{layer}_{hh}_{jt}")
                            nc.scalar.activation(z[:], pq[:], AF.Relu,
                                                 bias=neg1[:])
                            hs = hps_for(hh, jt)
                            mm[hh].append((z[:], hs[:]))
                            mm[hh].append((adj2[jt][:, 0:N], hs[:]))
                    else:
                        u2 = u_pool.tile([128, W2], BF16, tag="u",
                                         name=f"u_{g}_{layer}_{w}_{jt}")
                        # E''*ed2 = max(q_i*ed_j, ed2_j): one dual-op TS/head
                        nc.vector.tensor_scalar(
                            u2[:, 0:N], q_bc[w][:, 0:N],
                            ed_sc[jt][:, h0:h0 + 1], ed2_sc[jt][:, h0:h0 + 1],
                            AluOpType.mult, AluOpType.max)
                        nc.vector.tensor_scalar(
                            u2[:, N:W2], q_bc[w][:, N:W2],
                            ed_sc[jt][:, h1:h1 + 1], ed2_sc[jt][:, h1:h1 + 1],
                            AluOpType.mult, AluOpType.max)
                        et = et_pool.tile([128, W2], BF16, tag="et",
                                          name=f"et_{g}_{layer}_{w}_{jt}")
                        eng = nc.gpsimd if _mask_gps(w, jt) else nc.vector
                        eng.tensor_tensor(et[:], u2[:], adj2[jt][:],
                                          AluOpType.mult)
                        mm[h0].append((et[:, 0:N], hp_aug[jt][:]))
                        mm[h1].append((et[:, N:W2], hp_aug[jt][:]))

                for k, hh in enumerate((h0, h1)):
                    sl = slice(k * N, (k + 1) * N)
                    for i, (rhs, lhsT) in enumerate(mm[hh]):
                        nc.tensor.matmul(po[:, sl], lhsT, rhs,
                                         start=(i == 0),
                                         stop=(i == len(mm[hh]) - 1))

                # evac: 65 rows (den rides along as row 64, free on ACT)
                xew = xe_pool.tile([128, N], BF16, tag="xe",
                                   name=f"xe_{g}_{layer}_{w}")
                nc.scalar.copy(xew[0:F + 1, :], po[0:F + 1, 0:N])
                nc.sync.dma_start(dnd[h0:h0 + 1, :], xew[F:F + 1, :])
                scr = scr_pool.tile([F + 1, N], BF16, tag="scr",
                                    name=f"scr_{g}_{layer}_{w}")
                nc.scalar.copy(scr[:], po[0:F + 1, N:W2])
                nc.sync.dma_start(dnd[h1:h1 + 1, :], scr[F:F + 1, :])
                nc.sync.dma_start(xew[F:128, :], scr[0:F, :])
                xe.append(xew)

            # ---- normalization scales (reciprocal runs wide on [128, 32]) ----
            nc.sync.dma_start(
                den128[:], dnd.rearrange("h (p a) -> (h p) a", a=32))
            rd128 = den_pool.tile([128, 32], BF16, tag="rd128",
                                  name=f"rd_{g}_{layer}")
            with nc.allow_low_precision(reason="1/den in bf16 is ample"):
                nc.vector.reciprocal(rd128[:], den128[:])
            rddram = dbnc_pool.tile([H, N], BF16, tag="rddram",
                                    name=f"rddram_{g}_{layer}")
            nc.sync.dma_start(
                rddram.rearrange("h (p a) -> (h p) a", a=32), rd128[:])
            rdbc = []
            for ct in range(NT):
                rb = rdbc_pool.tile([128, N], BF16, tag="rdbc",
                                    name=f"rdbc_{g}_{layer}_{ct}")
                nc.sync.dma_start(
                    rb[0:F, :],
                    rddram[2 * ct:2 * ct + 1, :].partition_broadcast(F))
                nc.sync.dma_start(
                    rb[F:128, :],
                    rddram[2 * ct + 1:2 * ct + 2, :].partition_broadcast(F))
                rdbc.append(rb)
            return dict(xe=xe, rdbc=rdbc)

        def post(g, layer, ectx):
            xe, rdbc = ectx["xe"], ectx["rdbc"]
            if layer == 0:
                x1t = [x1t_pool.tile([128, N], BF16, tag="x1t",
                                     name=f"x1t_{g}_{ct}")
                       for ct in range(NT)]
                graphs[g]["x1t"] = x1t
                for ct in range(NT):
                    xn = post_pool.tile([128, N], BF16, tag="xn",
                                        name=f"xn_{g}_{ct}")
                    nc.vector.tensor_tensor(xn[:], xe[ct][:], rdbc[ct][:],
                                            AluOpType.mult)
                    te = post_pool.tile([128, N], BF16, tag="te",
                                        name=f"te_{g}_{ct}")
                    nc.scalar.activation(te[:], xn[:], AF.Exp)
                    rl = post_pool.tile([128, N], BF16, tag="rl",
                                        name=f"rl_{g}_{ct}")
                    nc.scalar.activation(rl[:], xn[:], AF.Relu)
                    # elu(x) = min(relu(x), exp(x) - 1)
                    nc.vector.scalar_tensor_tensor(
                        x1t[ct][:], te[:], -1.0, rl[:],
                        AluOpType.add, AluOpType.min)
            else:
                x2t = []
                for ct in range(NT):
                    xn = x2t_pool.tile([128, N], BF16, tag="x2t",
                                       name=f"x2t_{g}_{ct}")
                    nc.vector.tensor_tensor(xn[:], xe[ct][:], rdbc[ct][:],
                                            AluOpType.mult)
                    x2t.append(xn)
                oo = out_pool.tile([128, NT * F], F32, tag="oo",
                                   name=f"oo_{g}")
                for it in range(NT):
                    po2 = ps_qr.tile([128, F], F32, tag="qr",
                                     name=f"po2_{g}_{it}")
                    for ct in range(NT):
                        nc.tensor.matmul(
                            po2[:], x2t[ct][:, it * 128:(it + 1) * 128],
                            sel2_sb[:], start=(ct == 0), stop=(ct == NT - 1))
                    nc.scalar.copy(oo[:, it * F:(it + 1) * F], po2[:])
                nc.sync.dma_start(
                    out[g].rearrange("(it p) f -> p it f", p=128),
                    oo[:].rearrange("p (it f) -> p it f", it=NT))

        # software-pipelined emission: unit k+1's prologue lands before unit
        # k's main body, and unit k's post (which waits on the den/reciprocal
        # DMA chain) lands after unit k+1's E-build, so in-order engines
        # don't head-of-line block on semaphore waits
        U = [(0, 0), (1, 0), (2, 0), (0, 1), (3, 0), (1, 1), (2, 1), (3, 1)]
        pending = {U[0]: prologue(*U[0])}
        prev = None
        for i, u in enumerate(U):
            if i + 1 < len(U):
                nxt = U[i + 1]
                pending[nxt] = prologue(*nxt)
            ectx = main(u[0], u[1], pending.pop(u))
            if prev is not None:
                post(prev[0][0], prev[0][1], prev[1])
            prev = (u, ectx)
        post(prev[0][0], prev[0][1], prev[1])


def _get_nc():
    if "nc" not in _cached:
        _cached["nc"] = _build()
    return _cached["nc"]


def _prep_inputs(x, adj, W0, a_src0, a_dst0, W1, a_src1, a_dst1):
    x = np.asarray(x, np.float32)
    adj = np.array(adj, np.float32, copy=True)
    idx = np.arange(N)
    adj[:, idx, idx] = 1.0  # self loops (reference mutates adj the same way)
    xT = np.ascontiguousarray(x.transpose(0, 2, 1))          # [B, 64, 512]
    adjPf = np.where(adj > 0, np.float32(1), np.float32(0)).astype(BF)
    adjMf = np.where(adj > 0, np.float32(0), np.float32(-BIG)).astype(BF)
    identf = np.eye(128, dtype=np.float32).astype(BF)
    W0 = np.asarray(W0, np.float32)
    W1 = np.asarray(W1, np.float32)
    # column layout: [W | W@a_dst | W@a_src] -> pp = [hp | d | s]
    w0d = np.concatenate(
        [W0, W0 @ np.asarray(a_dst0, np.float32),
         W0 @ np.asarray(a_src0, np.float32)], axis=1)
    w1d = np.concatenate(
        [W1, W1 @ np.asarray(a_dst1, np.float32),
         W1 @ np.asarray(a_src1, np.float32)], axis=1).astype(BF)
    sel2f = np.tile(np.eye(F, dtype=np.float32), (2, 1)).astype(BF)
    in_maps = []
    for c in range(NCORES):
        sl = slice(c * G, (c + 1) * G)
        in_maps.append(dict(
            xT=np.ascontiguousarray(xT[sl]),
            adjP=np.ascontiguousarray(adjPf[sl]),
            adjM=np.ascontiguousarray(adjMf[sl]),
            ident=identf, w0d=w0d, w1d=w1d, sel2=sel2f,
        ))
    return in_maps


def run(inputs, **kw):
    """Build+run; returns (output [B,N,F] float32, BassKernelResults)."""
    nc = _get_nc()
    in_maps = _prep_inputs(
        inputs["x"], inputs["adj"], inputs["W0"], inputs["a_src0"],
        inputs["a_dst0"], inputs["W1"], inputs["a_src1"], inputs["a_dst1"])
    res = run_bass_kernel_spmd(nc, in_maps, list(range(NCORES)), **kw)
    outs = [res.results[c]["out"].reshape(G, N, F) for c in range(NCORES)]
    return np.concatenate(outs, axis=0).astype(np.float32), res


def kernel(**inputs):
    out, _ = run(inputs)
    return out
